# revision 44
# baseline (speedup 1.0000x reference)
"""Trainium2 Bass kernel for nn_AttentionAggregator (gnn_message_passing).

Two SPMD launches over 8 NeuronCores, data-parallel over nodes (512 users +
512 items per core), with a tiny host relay between them.

Key ideas:
  - Algebraic reorder: relu(softmax(Q K^T) @ C @ W) == relu(softmax(Q K^T) @ (C @ W)),
    shrinking the dominant matmul from [4096,4096]@[4096,2048] to
    [4096,4096]@[4096,128] (~9x fewer FLOPs).
  - Launch 1 runs the data-dependent row gathers on GPSIMD SWDGE; the review
    table is gathered as bf16 4-row 512B blocks (block id fits int16; halves
    the bytes of the fp32 1KB-block variant) followed by a predicated 4-way
    select at 2x DVE rate; item/user rows gather directly as fp32 256B rows.
    Gathers are issued as 512-index sub-calls rotated over 4 SWDGE queues so
    transfers drain concurrently while the Pool engine issues ahead.
  - Under the gather shadow: S^T = K q^T on the tensor engine (fp16 tables,
    two K=64 matmuls packed into disjoint PE row groups) and exp(S/8) on
    ScalarE (scores ~N(0,1): no max subtraction), E^T out to DRAM in bf16;
    gathered tiles are pair-transposed (bf16 reviews, fp32 items) and
    projected against j-pair-packed bf16 weights -> h blocks [512,128].
  - Launch 2: PV matmul with bf16 E^T stationary against the host-relayed
    all-core [h | 1] bf16 table (fused row-sum column), reciprocal-normalize
    + relu.
"""

import sys

for _p in ("/opt/trn_rl_repo",):
    if _p not in sys.path:
        sys.path.append(_p)

import numpy as np

import concourse.bacc as bacc
import concourse.mybir as mybir
import concourse.tile as tile
from concourse.bass_utils import run_bass_kernel_spmd
from concourse.masks import make_identity

F32 = mybir.dt.float32
BF16 = mybir.dt.bfloat16
FP16 = mybir.dt.float16
I16 = mybir.dt.int16
I8 = mybir.dt.int8
AF = mybir.ActivationFunctionType

N_REV, NU, DEG, D, HID = 100000, 4096, 16, 64, 128
N_CORES = 8
UB = NU // N_CORES          # 512 rows per core per side
NT = UB // 128              # 4 node tiles per core per side
NSLOT = NT * DEG            # 64 gathered slots per side (c = t*16 + j)
NG = UB * DEG               # 8192 gathered rows per table per side
TG = NG // NT               # 2048 gathered rows per node tile
MT = NU // 128              # 32 m tiles
QB = UB
QT = QB // 128
G = 2                       # m-tiles per QK/exp group
BLK = 4                     # review rows per gathered block
SCALE = 1.0 / float(np.sqrt(D))


def _build_k1():
    nc = bacc.Bacc("TRN2", target_bir_lowering=False, debug=False,
                   enable_asserts=True, num_devices=N_CORES,
                   num_swdge_queues=4)
    revb = nc.dram_tensor("revb", [N_REV // BLK, BLK * D], BF16, kind="ExternalInput")
    # item/user tables as bf16 2-row 256B blocks (block id fits int16 trivially)
    usert = nc.dram_tensor("usert", [NU // 2, 2 * D], BF16, kind="ExternalInput")
    itemt = nc.dram_tensor("itemt", [NU // 2, 2 * D], BF16, kind="ExternalInput")
    # weights, j-pair packed: [128 = (j%2)*64+d, DEG//2, HID]
    wr_u = nc.dram_tensor("wr_u", [128, DEG // 2, HID], BF16, kind="ExternalInput")
    wr_i = nc.dram_tensor("wr_i", [128, DEG // 2, HID], BF16, kind="ExternalInput")
    wd_u = nc.dram_tensor("wd_u", [128, DEG // 2, HID], BF16, kind="ExternalInput")
    wd_i = nc.dram_tensor("wd_i", [128, DEG // 2, HID], BF16, kind="ExternalInput")
    bidx = nc.dram_tensor("bidx", [2, 128, NG // 16], I16, kind="ExternalInput")
    iidx = nc.dram_tensor("iidx", [2, 128, NG // 16], I16, kind="ExternalInput")
    selm = nc.dram_tensor("selm", [2, 128, NSLOT, BLK], I8, kind="ExternalInput")
    selo = nc.dram_tensor("selo", [2, 128, NSLOT, 2], I8, kind="ExternalInput")
    vtu = nc.dram_tensor("vtu", [D, NU], FP16, kind="ExternalInput")
    vtuq = nc.dram_tensor("vtuq", [D, QB], FP16, kind="ExternalInput")
    vti = nc.dram_tensor("vti", [D, NU], FP16, kind="ExternalInput")
    vtiq = nc.dram_tensor("vtiq", [D, QB], FP16, kind="ExternalInput")
    hu = nc.dram_tensor("hu", [UB, HID], F32, kind="ExternalOutput")
    hi = nc.dram_tensor("hi", [UB, HID], F32, kind="ExternalOutput")
    et = nc.dram_tensor("et", [2, MT, 128, QB], BF16, kind="ExternalOutput")

    with tile.TileContext(nc) as tc:
        with (
            tc.tile_pool(name="singles", bufs=1) as singles,
            tc.tile_pool(name="vtp", bufs=1) as vtp,
            tc.tile_pool(name="etp", bufs=3) as etp,
            tc.tile_pool(name="stgp", bufs=3) as stgp,
            tc.tile_pool(name="xip", bufs=2) as xip,
            tc.tile_pool(name="xrp", bufs=2) as xrp,
            tc.tile_pool(name="xtp", bufs=2) as xtp,
            tc.tile_pool(name="outb", bufs=4) as outb,
            tc.tile_pool(name="sps", bufs=2, space="PSUM") as sps,
            tc.tile_pool(name="tps", bufs=1, space="PSUM") as tps,
            tc.tile_pool(name="hps", bufs=1, space="PSUM") as hps,
        ):
            bidx_sb = singles.tile([128, 2, NG // 16], I16)
            nc.sync.dma_start(out=bidx_sb[:], in_=bidx.ap().rearrange("a p s -> p a s"))
            iidx_sb = singles.tile([128, 2, NG // 16], I16)
            nc.sync.dma_start(out=iidx_sb[:], in_=iidx.ap().rearrange("a p s -> p a s"))
            selm_sb = singles.tile([128, 2, NSLOT, BLK], I8)
            nc.sync.dma_start(out=selm_sb[:], in_=selm.ap().rearrange("a p c b -> p a c b"))
            selo_sb = singles.tile([128, 2, NSLOT, 2], I8)
            nc.sync.dma_start(out=selo_sb[:], in_=selo.ap().rearrange("a p c b -> p a c b"))
            ident = singles.tile([128, 128], BF16)
            w_sb = {}
            for nm, t in (("wr_u", wr_u), ("wr_i", wr_i), ("wd_u", wd_u), ("wd_i", wd_i)):
                w = singles.tile([128, DEG // 2, HID], BF16, tag=nm, name=f"{nm}_sb")
                nc.scalar.dma_start(out=w[:], in_=t.ap())
                w_sb[nm] = w

            # ---- dense scores + exp -> E^T to DRAM (under the gather shadow)
            for side, (vt_d, vtq_d) in enumerate(((vtu, vtuq), (vti, vtiq))):
                vt_sb = vtp.tile([D, NU], FP16, tag="vt")
                vtq_sb = vtp.tile([D, QB], FP16, tag="vtq")
                nc.sync.dma_start(out=vt_sb[:], in_=vt_d.ap())
                nc.sync.dma_start(out=vtq_sb[:], in_=vtq_d.ap())
                for g in range(MT // G):
                    s_ps = sps.tile([128, G, QB], F32, tag="sps")
                    for k in range(G):
                        m = g * G + k
                        nc.tensor.matmul(
                            s_ps[:, k, :],
                            lhsT=vt_sb[:, m * 128:(m + 1) * 128],
                            rhs=vtq_sb[:],
                            start=True, stop=True,
                        )
                    etb = etp.tile([128, G, QB], BF16, tag="etb")
                    nc.scalar.activation(etb[:], s_ps[:], AF.Exp, scale=SCALE)
                    nc.sync.dma_start(
                        out=et.ap()[side, g * G:(g + 1) * G, :, :].rearrange("g p q -> p g q"),
                        in_=etb[:],
                    )

            # ---- gathers (512-idx SWDGE sub-calls rotated over 4 queues)
            # + select + transpose + project, pipelined per 128-node tile ----
            qctr = [0]
            for side, (otbl, wr, wd, hout) in enumerate((
                (itemt, "wr_u", "wd_u", hu),
                (usert, "wr_i", "wd_i", hi),
            )):
                for t in range(NT):
                    sl = slice(t * DEG, (t + 1) * DEG)
                    c0 = t * (TG // 16)     # idx column offset for this tile
                    CH = 1024               # idxs per SWDGE sub-call
                    stg = stgp.tile([128, DEG, BLK * D], BF16, tag="stg")
                    for s in range(TG // CH):
                        nc.gpsimd.dma_gather(
                            out_ap=stg[:, s * (CH // 128):(s + 1) * (CH // 128), :],
                            in_ap=revb.ap(),
                            idxs_ap=bidx_sb[:, side,
                                            c0 + s * (CH // 16):c0 + (s + 1) * (CH // 16)],
                            num_idxs=CH, num_idxs_reg=CH, elem_size=BLK * D,
                            single_packet=False, queue_num=qctr[0] % 4,
                        )
                        qctr[0] += 1
                    stgo = xip.tile([128, DEG, 2 * D], BF16, tag="stgo")
                    for s in range(TG // CH):
                        nc.gpsimd.dma_gather(
                            out_ap=stgo[:, s * (CH // 128):(s + 1) * (CH // 128), :],
                            in_ap=otbl.ap(),
                            idxs_ap=iidx_sb[:, side,
                                            c0 + s * (CH // 16):c0 + (s + 1) * (CH // 16)],
                            num_idxs=CH, num_idxs_reg=CH, elem_size=2 * D,
                            single_packet=False, queue_num=qctr[0] % 4,
                        )
                        qctr[0] += 1
                    if side == 0 and t == 0:
                        # identity is first needed by tile-0 transposes;
                        # emitting it here keeps the GPSIMD preamble off the
                        # first gather's critical path
                        make_identity(nc, ident[:])

                    xr = xrp.tile([128, DEG, D], BF16, tag="xr")
                    nc.vector.tensor_copy(xr[:], stg[:, :, 0:D])
                    for b in range(1, BLK):
                        mb = selm_sb[:, side, sl, b][:, :, None].broadcast_to([128, DEG, D])
                        nc.vector.copy_predicated(xr[:], mb, stg[:, :, b * D:(b + 1) * D])
                    xi = xrp.tile([128, DEG, D], BF16, tag="xi")
                    nc.vector.tensor_copy(xi[:], stgo[:, :, 0:D])
                    mo = selo_sb[:, side, sl, 1][:, :, None].broadcast_to([128, DEG, D])
                    nc.vector.copy_predicated(xi[:], mo, stgo[:, :, D:2 * D])

                    xtr = xtp.tile([128, DEG // 2, 128], BF16, tag="xtr")
                    xti = xtp.tile([128, DEG // 2, 128], BF16, tag="xti")
                    ps_r = tps.tile([128, 2, 128], BF16, tag="tpsr")
                    ps_i = tps.tile([128, 2, 128], BF16, tag="tpsi")
                    for q in range(DEG // 2):
                        nc.tensor.transpose(ps_r[:, q % 2, :],
                                            xr[:, 2 * q: 2 * q + 2, :], ident[:])
                        nc.vector.tensor_copy(xtr[:, q, :], ps_r[:, q % 2, :])
                        nc.tensor.transpose(ps_i[:, q % 2, :],
                                            xi[:, 2 * q: 2 * q + 2, :], ident[:])
                        nc.vector.tensor_copy(xti[:, q, :], ps_i[:, q % 2, :])

                    h_ps = hps.tile([128, HID], F32, tag="hps")
                    for q in range(DEG // 2):
                        nc.tensor.matmul(h_ps[:], lhsT=xtr[:, q, :], rhs=w_sb[wr][:, q, :],
                                         start=(q == 0), stop=False, skip_group_check=True)
                        nc.tensor.matmul(h_ps[:], lhsT=xti[:, q, :], rhs=w_sb[wd][:, q, :],
                                         start=False, stop=(q == DEG // 2 - 1),
                                         skip_group_check=True)
                    h_sb = outb.tile([128, HID], F32, tag="hsb")
                    nc.vector.tensor_copy(h_sb[:], h_ps[:])
                    nc.sync.dma_start(out=hout.ap()[t * 128:(t + 1) * 128, :], in_=h_sb[:])

    nc.compile()
    return nc


def _build_k2():
    nc = bacc.Bacc("TRN2", target_bir_lowering=False, debug=False,
                   enable_asserts=True, num_devices=N_CORES)
    et = nc.dram_tensor("et", [2, MT, 128, QB], BF16, kind="ExternalInput")
    hau = nc.dram_tensor("hau", [128, MT, HID + 1], BF16, kind="ExternalInput")
    hai = nc.dram_tensor("hai", [128, MT, HID + 1], BF16, kind="ExternalInput")
    uo = nc.dram_tensor("uo", [QB, HID], F32, kind="ExternalOutput")
    io = nc.dram_tensor("io", [QB, HID], F32, kind="ExternalOutput")

    with tile.TileContext(nc) as tc:
        with (
            tc.tile_pool(name="etp", bufs=2) as etp,
            tc.tile_pool(name="ha", bufs=2) as hap,
            tc.tile_pool(name="ob", bufs=4) as obp,
            tc.tile_pool(name="aps", bufs=1, space="PSUM") as aps,
        ):
            for side, (ha_d, out_d) in enumerate(((hau, uo), (hai, io))):
                et_sb = etp.tile([128, MT, QB], BF16, tag="et")
                CH = MT // 8
                for ch in range(8):
                    eng = nc.sync if ch % 2 == 0 else nc.scalar
                    eng.dma_start(
                        out=et_sb[:, ch * CH:(ch + 1) * CH, :],
                        in_=et.ap()[side, ch * CH:(ch + 1) * CH].rearrange("m p q -> p m q"))
                ha_sb = hap.tile([128, MT, HID + 1], BF16, tag="ha")
                nc.sync.dma_start(out=ha_sb[:], in_=ha_d.ap())

                att_ps = [aps.tile([128, HID + 1], F32, tag=f"att{qt}", name=f"att{qt}_{side}")
                          for qt in range(QT)]
                for m in range(MT):
                    for qt in range(QT):
                        nc.tensor.matmul(
                            att_ps[qt][:],
                            lhsT=et_sb[:, m, qt * 128:(qt + 1) * 128],
                            rhs=ha_sb[:, m, :],
                            start=(m == 0), stop=(m == MT - 1),
                            skip_group_check=True,
                        )
                for qt in range(QT):
                    recip = obp.tile([128, 1], F32, tag="recip")
                    nc.vector.reciprocal(recip[:], att_ps[qt][:, HID:HID + 1])
                    o_sb = obp.tile([128, HID], F32, tag="osb")
                    nc.scalar.activation(o_sb[:], att_ps[qt][:, 0:HID], AF.Relu,
                                         scale=recip[:, 0:1])
                    nc.sync.dma_start(out=out_d.ap()[qt * 128:(qt + 1) * 128, :], in_=o_sb[:])
    nc.compile()
    return nc


_CACHE = {}


def _programs():
    if "k1" not in _CACHE:
        _CACHE["k1"] = _build_k1()
        _CACHE["k2"] = _build_k2()
    return _CACHE["k1"], _CACHE["k2"]


def _arr(x, dt):
    return np.ascontiguousarray(np.asarray(x), dtype=dt)


def _wrap16(a):
    # flat int list -> [128, n/16] int16: index i at partition i%16, slot
    # i//16, replicated for the 8 Q7 cores
    a = np.asarray(a)
    return np.tile(a.reshape(-1, 16).T, (8, 1)).astype(np.int16)


def _cmajor(adj_blk):
    # [UB, DEG] -> flat vals[i], i = (t*DEG+j)*128 + p, node = t*128+p
    return adj_blk.reshape(NT, 128, DEG).transpose(0, 2, 1).reshape(-1)


def _split_w(w):
    # [DEG*2*D, HID] -> (rev-part, other-part) j-pair packed [128, DEG//2, HID] bf16
    import ml_dtypes
    w4 = w.reshape(DEG, 2, D, HID)

    def pack(part):
        return np.ascontiguousarray(
            part.reshape(DEG // 2, 128, HID).transpose(1, 0, 2)
            .astype(ml_dtypes.bfloat16))

    return pack(w4[:, 0].reshape(DEG * D, HID)), pack(w4[:, 1].reshape(DEG * D, HID))


def _aug_tiled(h):
    import ml_dtypes
    ha = np.concatenate([h, np.ones((NU, 1), np.float32)], axis=1)
    ha = ha.reshape(MT, 128, HID + 1).transpose(1, 0, 2)
    return np.ascontiguousarray(ha.astype(ml_dtypes.bfloat16))


def kernel(review_vecs, user_vecs, item_vecs, user_weights, item_weights,
           adj0, adj1, adj2, adj3, _profile=None):
    import ml_dtypes
    rev = _arr(review_vecs, np.float32)
    uv = _arr(user_vecs, np.float32)
    iv = _arr(item_vecs, np.float32)
    wu = _arr(user_weights, np.float32)
    wi = _arr(item_weights, np.float32)
    a0, a1, a2, a3 = (np.asarray(a).astype(np.int64) for a in (adj0, adj1, adj2, adj3))

    revb = np.ascontiguousarray(
        rev.astype(ml_dtypes.bfloat16).reshape(N_REV // BLK, BLK * D))
    ub2 = np.ascontiguousarray(uv.astype(ml_dtypes.bfloat16).reshape(NU // 2, 2 * D))
    ib2 = np.ascontiguousarray(iv.astype(ml_dtypes.bfloat16).reshape(NU // 2, 2 * D))
    wr_u, wd_u = _split_w(wu)
    wr_i, wd_i = _split_w(wi)
    uvt = np.ascontiguousarray(uv.T.astype(np.float16))
    ivt = np.ascontiguousarray(iv.T.astype(np.float16))

    k1, k2 = _programs()
    cores = list(range(N_CORES))

    in_maps1 = []
    for c in cores:
        bidx = np.zeros((2, 128, NG // 16), np.int16)
        iidx = np.zeros((2, 128, NG // 16), np.int16)
        selm = np.zeros((2, 128, NSLOT, BLK), np.int8)
        selo = np.zeros((2, 128, NSLOT, 2), np.int8)
        for side, (a_rev, a_oth) in enumerate(((a0, a1), (a2, a3))):
            rvals = _cmajor(a_rev[c * UB:(c + 1) * UB])
            ovals = _cmajor(a_oth[c * UB:(c + 1) * UB])
            bidx[side] = _wrap16(rvals // BLK)
            iidx[side] = _wrap16(ovals // 2)
            sel = (rvals % BLK).reshape(NSLOT, 128).T  # [p, c]
            for b in range(BLK):
                selm[side, :, :, b] = (sel == b)
            sel2 = (ovals % 2).reshape(NSLOT, 128).T
            for b in range(2):
                selo[side, :, :, b] = (sel2 == b)
        in_maps1.append({
            "revb": revb, "usert": ub2, "itemt": ib2,
            "wr_u": wr_u, "wr_i": wr_i, "wd_u": wd_u, "wd_i": wd_i,
            "bidx": bidx, "selm": selm, "iidx": iidx, "selo": selo,
            "vtu": uvt, "vtuq": np.ascontiguousarray(uvt[:, c * QB:(c + 1) * QB]),
            "vti": ivt, "vtiq": np.ascontiguousarray(ivt[:, c * QB:(c + 1) * QB]),
        })
    r1 = run_bass_kernel_spmd(k1, in_maps1, core_ids=cores, trace=_profile is not None)
    h_user = np.concatenate([r1.results[c]["hu"] for c in cores], axis=0)
    h_item = np.concatenate([r1.results[c]["hi"] for c in cores], axis=0)

    hau = _aug_tiled(h_user)
    hai = _aug_tiled(h_item)
    in_maps2 = [{
        "et": r1.results[c]["et"], "hau": hau, "hai": hai,
    } for c in cores]
    r2 = run_bass_kernel_spmd(k2, in_maps2, core_ids=cores, trace=_profile is not None)

    user_out = np.concatenate([r2.results[c]["uo"] for c in cores], axis=0)
    item_out = np.concatenate([r2.results[c]["io"] for c in cores], axis=0)

    if _profile is not None:
        _profile["k1"] = r1
        _profile["k2"] = r2
    return user_out, item_out
